# revision 26
# baseline (speedup 1.0000x reference)
"""Cross-covariance attention (XCA) kernel for Trainium2, 8 NeuronCores.

Problem (per batch element b, one per core — data-parallel over B=8):
    qkv = x @ Wqkv;  q,k,v heads of dim 64;  q,k L2-normalized over the
    TOKEN axis;  attn_h = softmax((k_h^T q_h) * temp_h) (64x64, head-local);
    y = concat_h(v_h @ attn_h) @ Wout + bout.

Key algebraic reduction: the attention matrix only depends on the token
covariance C = x^T x (768x768):
    k_h^T q_h = Wk_h^T C Wq_h,   ||q_col_j||^2 = diag(Wq^T C Wq)_j
and the output collapses to a single matmul with a precomputed 768x768
matrix:
    y = x @ W3 + bout,  W3 = Wv @ blockdiag(A_h) @ Wout.

Per core: C = x^T x (contraction over tokens => natural x layout; symmetric,
so only the upper block-triangle is computed and the rest is mirrored with
PE transposes), small 768-scale linear algebra to form W3, then y = x @ W3
(x transposed 128x128-blockwise on the PE on the fly).

Precision strategy (validated numerically): the C / final-matmul paths run
in bf16 (PE 1 cyc/row + fast weight load; softmax smooths C errors to
~3e-5), while the norm/logits path (Mqk, norms, G) stays in
float32r/fp32. PSUM accumulation is always fp32.
"""

import os
import sys

sys.path.insert(0, "/opt/trn_rl_repo")

import numpy as np

import concourse.bacc as bacc
import concourse.bass as bass
import concourse.mybir as mybir
import concourse.tile as tile
from concourse.bass_utils import run_bass_kernel_spmd
from concourse.masks import make_identity

F32 = mybir.dt.float32
F32R = mybir.dt.float32r
# fp16 (not bf16): same PE rate (1 cyc/row + FWL) but 10 mantissa bits;
# all tensors in this problem are O(10^3) max, far inside fp16 range
BF16 = mybir.dt.float16

B, N, D = 8, 4096, 768
H, DH = 12, 64
P = 128
KT = D // P  # 6 feature tiles
TT = N // P  # 32 token tiles
HP = H // 2  # 6 head pairs (2 heads packed into 128 partitions)
EPS = 1e-12
NPREF = 14  # token tiles whose transposes are hoisted into phase C/D gaps

# bf16 for the final y = x @ W3 matmul (adds ~2.8e-3 absmax-relative error;
# set XCA_BF16_E=0 to run that phase in float32r instead)
BF16_E = os.environ.get("XCA_BF16_E", "1") == "1"

if os.environ.get("BASS_LDW_OPT", "0") == "1":
    # Allow walrus to dedup back-to-back LDWEIGHTS with identical sources
    # (bass passes --enable-ldw-opt=false by default). Loop orders below are
    # arranged so consecutive matmuls share their stationary operand.
    import concourse.bass_utils as _bu

    if not getattr(_bu, "_ldw_opt_patched", False):
        _orig_run_command = _bu.run_command

        def _run_command_ldw(argv, **kwargs):
            argv = [
                "--enable-ldw-opt=true" if a == "--enable-ldw-opt=false" else a
                for a in argv
            ]
            return _orig_run_command(argv, **kwargs)

        _bu.run_command = _run_command_ldw
        _bu._ldw_opt_patched = True


def build_nc():
    nc = bacc.Bacc("TRN2", target_bir_lowering=False, debug=False)

    x_d = nc.dram_tensor("x", (N, D), F32, kind="ExternalInput")
    wqkv_d = nc.dram_tensor("wqkv", (D, 3 * D), F32, kind="ExternalInput")
    temp_d = nc.dram_tensor("temp", (H,), F32, kind="ExternalInput")
    wout_d = nc.dram_tensor("wout", (D, D), F32, kind="ExternalInput")
    bout_d = nc.dram_tensor("bout", (D,), F32, kind="ExternalInput")
    y_d = nc.dram_tensor("y", (N, D), F32, kind="ExternalOutput")

    with tile.TileContext(nc) as tc:
        _emit(tc, nc, x_d, wqkv_d, temp_d, wout_d, bout_d, y_d)
    nc.compile()
    return nc


def _emit(tc, nc, x_d, wqkv_d, temp_d, wout_d, bout_d, y_d):
    from contextlib import ExitStack

    E_DT = BF16

    ctx = ExitStack()
    with ctx:
        # ---------------- pools ----------------
        persist = ctx.enter_context(tc.tile_pool(name="persist", bufs=1))
        bigpool = ctx.enter_context(tc.tile_pool(name="bigpool", bufs=1))
        xbpool = ctx.enter_context(tc.tile_pool(name="xbpool", bufs=6))
        small = ctx.enter_context(tc.tile_pool(name="small", bufs=1))
        tmppool = ctx.enter_context(tc.tile_pool(name="tmppool", bufs=2))
        ypool = ctx.enter_context(tc.tile_pool(name="ypool", bufs=3))
        xtpool = ctx.enter_context(tc.tile_pool(name="xtpool", bufs=NPREF + 4))
        abd_pool = ctx.enter_context(tc.tile_pool(name="abd", bufs=6))

        wqk_sb = persist.tile([P, KT, 2 * D], F32R)  # Wqkv[:, :1536]
        wqk_bf = persist.tile([P, KT, 2 * D], BF16)  # bf16 copy for C @ [Wq|Wk]
        mq_sb = persist.tile([P, KT, D], F32R)  # C @ Wq (later scaled by 1/nq)
        w2_sb = persist.tile([P, KT, D], E_DT)  # blockdiag(A) @ Wout
        wvt_sb = persist.tile([P, KT, D], E_DT)  # Wv^T
        w3_bf = persist.tile([P, KT, D], E_DT)  # W3 in phase-E dtype

        # prime the x stream before any other SWDGE work so the PE starts
        # as early as possible
        xb_head = []
        for t in range(3):
            xb0 = xbpool.tile([P, D], BF16, tag="xb", name="xb")
            nc.gpsimd.dma_start(xb0, x_d[t * P : (t + 1) * P, :])
            xb_head.append(xb0)

        # memsets reject f32r at codegen: build f32 and DVE-cast
        ident32 = small.tile([P, P], F32)
        make_identity(nc, ident32)
        ident = small.tile([P, P], F32R)
        nc.vector.tensor_copy(ident, ident32)
        identb = small.tile([P, P], BF16)
        nc.vector.tensor_copy(identb, ident32)
        ones32 = small.tile([P, P], F32)
        nc.vector.memset(ones32, 1.0)
        ones = small.tile([P, P], F32R)
        nc.vector.tensor_copy(ones, ones32)
        temp_sb = small.tile([P, H], F32)
        nc.gpsimd.dma_start(temp_sb, temp_d[None, :].to_broadcast((P, H)))
        bout_sb = small.tile([P, D], F32)
        nc.gpsimd.dma_start(bout_sb, bout_d[None, :].to_broadcast((P, D)))
        # s / rvec are replicated across all 128 partitions (the norm matmul
        # uses an all-ones [P,P] lhsT, so every output partition holds the sum)
        s_sb = small.tile([P, 2 * D], F32)  # [1/max(nq,eps) | 1/max(nk,eps)]
        rvec = small.tile([P, D], F32)  # temp_h / nk

        c_sb = bigpool.tile([P, KT, D], BF16, tag="big")

        def load_x_bf16(t):
            """One token tile of x, cast fp32->bf16 during the DMA (SWDGE)."""
            xb = xbpool.tile([P, D], BF16, tag="xb", name="xb")
            nc.gpsimd.dma_start(xb, x_d[t * P : (t + 1) * P, :])
            return xb

        # ------------- phase A: C = x^T x in bf16, upper block-triangle -----
        # row-block i covers cols [128*i, 768): 8 matmuls per token tile,
        # exactly 8 PSUM banks
        with tc.tile_pool(name="psC", bufs=1, space="PSUM") as psC:
            cps = [
                psC.tile([P, D - 128 * i], F32, name=f"cps{i}") for i in range(KT)
            ]
            for t in range(TT):
                xb = xb_head[t] if t < 3 else load_x_bf16(t)
                for i in range(KT):
                    w = D - 128 * i
                    for lo in range(0, w, 512):
                        hi = min(lo + 512, w)
                        nc.tensor.matmul(
                            cps[i][:, lo:hi],
                            xb[:, i * P : (i + 1) * P],
                            xb[:, 128 * i + lo : 128 * i + hi],
                            start=(t == 0),
                            stop=(t == TT - 1),
                        )
            for i in range(KT):
                nc.vector.tensor_copy(c_sb[:, i, 128 * i : D], cps[i])

        # weight loads (scalar HWDGE queue; does not block the x stream)
        nc.scalar.dma_start(
            wqk_sb,
            wqkv_d[:, 0 : 2 * D].rearrange("(ko p) c -> p ko c", p=P).bitcast(F32R),
        )
        nc.gpsimd.dma_start(
            wqk_bf, wqkv_d[:, 0 : 2 * D].rearrange("(ko p) c -> p ko c", p=P)
        )

        # shared PSUM pool for all 128x128 PE transposes (phases C..E)
        psTP = ctx.enter_context(tc.tile_pool(name="psTP", bufs=2, space="PSUM"))

        # mirror the lower block-triangle: block(j,i) = block(i,j)^T
        for i in range(KT):
            for j in range(i + 1, KT):
                tpm = psTP.tile([P, P], BF16, tag="tp", name="tpm")
                nc.tensor.transpose(tpm, c_sb[:, i, j * P : (j + 1) * P], identb)
                nc.vector.tensor_copy(c_sb[:, j, i * P : (i + 1) * P], tpm)

        # phase-E prefetch: transpose the first NPREF token tiles now, so the
        # PE has work during the DVE/ACT-heavy normalization phase
        xtt_tiles = {}

        def transpose_tile(t):
            xb = load_x_bf16(t)
            xtt = xtpool.tile([P, KT, P], BF16, tag="xtt", name="xtt")
            for k in range(KT):
                tpe = psTP.tile([P, P], BF16, tag="tp", name="tpe")
                nc.tensor.transpose(tpe, xb[:, k * P : (k + 1) * P], identb)
                nc.vector.tensor_copy(xtt[:, k, :], tpe)
            return xtt

        # ---------------- phase C: Mqk = C @ [Wq|Wk], norms -----------------
        with tc.tile_pool(name="psMQ", bufs=2, space="PSUM") as psMQ, tc.tile_pool(
            name="psN", bufs=1, space="PSUM"
        ) as psN:
            nrm_ps = psN.tile([P, 2 * D], F32)  # [nq^2 | nk^2], replicated
            for f in range(KT):
                mk_tmp = tmppool.tile([P, D], F32R, tag="mk", name="mk_tmp")
                for nch in range(3):
                    pmq = psMQ.tile([P, 512], F32, tag="pmq", name="pmq")
                    for k in range(KT):
                        nc.tensor.matmul(
                            pmq,
                            c_sb[:, k, f * P : (f + 1) * P],
                            wqk_bf[:, k, nch * 512 : (nch + 1) * 512],
                            start=(k == 0),
                            stop=(k == KT - 1),
                        )
                    if nch == 0:
                        nc.vector.tensor_copy(mq_sb[:, f, 0:512], pmq)
                    elif nch == 1:
                        nc.vector.tensor_copy(mq_sb[:, f, 512:768], pmq[:, 0:256])
                        nc.vector.tensor_copy(mk_tmp[:, 0:256], pmq[:, 256:512])
                    else:
                        nc.vector.tensor_copy(mk_tmp[:, 256:768], pmq)
                # norm partials: nq_j = sum_f Wq[f,j]*Mq[f,j] (and nk likewise)
                wt = tmppool.tile([P, 2 * D], F32R, tag="wt", name="wt")
                nc.vector.tensor_mul(wt[:, 0:D], wqk_sb[:, f, 0:D], mq_sb[:, f, :])
                nc.vector.tensor_mul(wt[:, D : 2 * D], wqk_sb[:, f, D : 2 * D], mk_tmp)
                for lo in range(0, 2 * D, 512):
                    nc.tensor.matmul(
                        nrm_ps[:, lo : lo + 512],
                        ones,
                        wt[:, lo : lo + 512],
                        start=(f == 0),
                        stop=(f == KT - 1),
                    )
                # interleave phase-E prefetch transposes with the Mqk flow
                for tpre in range(f * 2, f * 2 + 2):
                    xtt_tiles[tpre] = transpose_tile(tpre)
            # s = 1 / max(sqrt(nrm2), eps)
            nc.vector.tensor_copy(s_sb, nrm_ps)
        nc.scalar.sqrt(s_sb, s_sb)
        nc.vector.tensor_scalar_max(s_sb, s_sb, EPS)
        nc.vector.reciprocal(s_sb, s_sb)

        # rvec[h*64+p] = temp[h] * s_k[h*64+p]   (replicated on all partitions)
        rv3 = rvec.rearrange("o (h e) -> o h e", h=H)
        nc.vector.tensor_mul(
            rv3,
            s_sb[:, D : 2 * D].rearrange("o (h e) -> o h e", h=H),
            temp_sb[:, :, None].to_broadcast((P, H, DH)),
        )
        # scale Wk in place by rvec (rows=f, cols=(h,p)); Mq in place by 1/nq
        wk_view = wqk_sb[:, :, D : 2 * D]
        nc.vector.tensor_mul(
            wk_view, wk_view, rvec[:, None, :].to_broadcast((P, KT, D))
        )
        mq_view = mq_sb[:, :, :]
        nc.vector.tensor_mul(
            mq_view, mq_view, s_sb[:, None, 0:D].to_broadcast((P, KT, D))
        )

        for tpre in range(12, NPREF):
            xtt_tiles[tpre] = transpose_tile(tpre)

        # ---------------- phase D1: logits G, softmax -----------------------
        # one 128-wide matmul per head PAIR: off-diagonal 64x64 blocks are
        # cross-head garbage that is simply never read
        with tc.tile_pool(name="psG", bufs=1, space="PSUM") as psG:
            gps = [psG.tile([P, 2, P], F32, name=f"gps{i}") for i in range(3)]
            for hp in range(HP):
                out = gps[hp // 2][:, hp % 2, :]
                for f in range(KT):
                    nc.tensor.matmul(
                        out,
                        wqk_sb[:, f, D + hp * P : D + (hp + 1) * P].bitcast(F32),
                        mq_sb[:, f, hp * P : (hp + 1) * P].bitcast(F32),
                        start=(f == 0),
                        stop=(f == KT - 1),
                    )
            # per-head softmax written into the block-diag tile a_bd =
            # [[A_even, 0], [0, A_odd]]; one transpose per pair then gives
            # blockdiag(A_e^T, A_o^T) for the W2 matmul (base-0, K=128)
            abd_tiles = []
            for hp in range(HP):
                a_bd = abd_pool.tile([P, P], F32, tag="a_bd", name="a_bd")
                nc.vector.memset(a_bd, 0.0)
                for half in range(2):
                    lo64 = half * 64
                    g_view = gps[hp // 2][lo64 : lo64 + 64, hp % 2, lo64 : lo64 + 64]
                    negmax = small.tile([P, 1], F32, tag="negmax", name="negmax", bufs=4)
                    sumexp = small.tile([P, 1], F32, tag="sumexp", name="sumexp", bufs=4)
                    rec = small.tile([P, 1], F32, tag="rec", name="rec", bufs=4)
                    nm, se, rc = (
                        negmax[lo64 : lo64 + 64],
                        sumexp[lo64 : lo64 + 64],
                        rec[lo64 : lo64 + 64],
                    )
                    a_blk = a_bd[lo64 : lo64 + 64, lo64 : lo64 + 64]
                    nc.vector.tensor_reduce(
                        nm, g_view, axis=mybir.AxisListType.X,
                        op=mybir.AluOpType.max, negate=True,
                    )
                    nc.scalar.activation(
                        a_blk, g_view, mybir.ActivationFunctionType.Exp,
                        bias=nm, accum_out=se,
                    )
                    nc.vector.reciprocal(rc, se)
                    nc.vector.tensor_scalar_mul(a_blk, a_blk, rc)
                abd_tiles.append(a_bd)

        # ---------------- phase D2: W2 = blockdiag(A) @ Wout ----------------
        with tc.tile_pool(name="psW2", bufs=2, space="PSUM") as psW2:
            for hp in range(HP):
                tps = psTP.tile([P, P], F32, tag="tp", name="tps")
                nc.tensor.transpose(tps, abd_tiles[hp], ident32)
                at_bd = tmppool.tile([P, P], F32R, tag="at_bd", name="at_bd")
                nc.vector.tensor_copy(at_bd, tps)
                wout_pair = tmppool.tile([P, D], F32R, tag="woutp", name="wout_pair")
                nc.scalar.dma_start(
                    wout_pair, wout_d[hp * P : (hp + 1) * P, :].bitcast(F32R)
                )
                w2ps = psW2.tile([P, D], F32, tag="w2ps", name="w2ps")
                for lo, hi in ((0, 512), (512, 768)):
                    nc.tensor.matmul(
                        w2ps[:, lo:hi], at_bd, wout_pair[:, lo:hi],
                        start=True, stop=True,
                    )
                nc.vector.tensor_copy(w2_sb[:, hp, :], w2ps)
            # Wv^T via PE transposes (Wv streamed in 128-col stripes, cast to
            # the phase-E dtype during the DVE copy off PSUM)
            for ci in range(KT):
                wv_st = tmppool.tile([P, KT, P], BF16, tag="wvst", name="wv_st")
                nc.gpsimd.dma_start(
                    wv_st,
                    wqkv_d[:, 2 * D + ci * P : 2 * D + (ci + 1) * P].rearrange(
                        "(ko p) c -> p ko c", p=P
                    ),
                )
                for kf in range(KT):
                    t2 = psTP.tile([P, P], BF16, tag="tp", name="t2")
                    nc.tensor.transpose(t2, wv_st[:, kf, :], identb)
                    nc.vector.tensor_copy(wvt_sb[:, ci, kf * P : (kf + 1) * P], t2)

        # ---------------- phase D3: W3 = Wv @ W2 (phase-E dtype) ------------
        with tc.tile_pool(name="psW3", bufs=2, space="PSUM") as psW3:
            for fi in range(KT):
                w3ps = psW3.tile([P, D], F32, tag="w3ps", name="w3ps")
                for g in range(KT):
                    for lo, hi in ((0, 512), (512, 768)):
                        nc.tensor.matmul(
                            w3ps[:, lo:hi],
                            wvt_sb[:, g, fi * P : (fi + 1) * P],
                            w2_sb[:, g, lo:hi],
                            start=(g == 0),
                            stop=(g == KT - 1),
                        )
                nc.vector.tensor_copy(w3_bf[:, fi, :], w3ps)

        # ---------------- phase E: y = x @ W3 + bout ------------------------
        with tc.tile_pool(name="psY", bufs=3, space="PSUM") as psY:
            for t in range(TT):
                xtt = xtt_tiles.pop(t, None)
                if xtt is None:
                    xtt = transpose_tile(t)
                yps = psY.tile([P, D], F32, tag="yps", name="yps")
                for k in range(KT):
                    for lo, hi in ((0, 512), (512, 768)):
                        nc.tensor.matmul(
                            yps[:, lo:hi],
                            xtt[:, k, :],
                            w3_bf[:, k, lo:hi],
                            start=(k == 0),
                            stop=(k == KT - 1),
                        )
                ysb = ypool.tile([P, D], F32, tag="ysb", name="ysb")
                nc.vector.tensor_add(ysb, yps, bout_sb)
                nc.sync.dma_start(y_d[t * P : (t + 1) * P, :], ysb)


_NC_CACHE = {}


def _get_nc():
    if "nc" not in _NC_CACHE:
        _NC_CACHE["nc"] = build_nc()
    return _NC_CACHE["nc"]


def kernel_with_results(x, Wqkv, temperature, Wout, bout, **run_kwargs):
    x = np.ascontiguousarray(np.asarray(x, dtype=np.float32))
    Wqkv = np.ascontiguousarray(np.asarray(Wqkv, dtype=np.float32))
    temp = np.ascontiguousarray(np.asarray(temperature, dtype=np.float32).reshape(H))
    Wout = np.ascontiguousarray(np.asarray(Wout, dtype=np.float32))
    bout = np.ascontiguousarray(np.asarray(bout, dtype=np.float32))

    nc = _get_nc()
    in_maps = [
        {"x": x[b], "wqkv": Wqkv, "temp": temp, "wout": Wout, "bout": bout}
        for b in range(B)
    ]
    res = run_bass_kernel_spmd(nc, in_maps, core_ids=list(range(B)), **run_kwargs)
    out = np.stack([r["y"] for r in res.results], axis=0)
    return out, res


def kernel(x, Wqkv, temperature, Wout, bout):
    out, _ = kernel_with_results(x, Wqkv, temperature, Wout, bout)
    return out


# revision 27
# speedup vs baseline: 1.0216x; 1.0216x over previous
"""Cross-covariance attention (XCA) kernel for Trainium2, 8 NeuronCores.

Problem (per batch element b, one per core — data-parallel over B=8):
    qkv = x @ Wqkv;  q,k,v heads of dim 64;  q,k L2-normalized over the
    TOKEN axis;  attn_h = softmax((k_h^T q_h) * temp_h) (64x64, head-local);
    y = concat_h(v_h @ attn_h) @ Wout + bout.

Key algebraic reduction: the attention matrix only depends on the token
covariance C = x^T x (768x768):
    k_h^T q_h = Wk_h^T C Wq_h,   ||q_col_j||^2 = diag(Wq^T C Wq)_j
and the output collapses to a single matmul with a precomputed 768x768
matrix:
    y = x @ W3 + bout,  W3 = Wv @ blockdiag(A_h) @ Wout.

Per core: C = x^T x (contraction over tokens => natural x layout; symmetric,
so only the upper block-triangle is computed and the rest is mirrored with
PE transposes), small 768-scale linear algebra to form W3, then y = x @ W3
(x transposed 128x128-blockwise on the PE on the fly).

Precision strategy (validated numerically): the C / final-matmul paths run
in bf16 (PE 1 cyc/row + fast weight load; softmax smooths C errors to
~3e-5), while the norm/logits path (Mqk, norms, G) stays in
float32r/fp32. PSUM accumulation is always fp32.
"""

import os
import sys

sys.path.insert(0, "/opt/trn_rl_repo")

import numpy as np

import concourse.bacc as bacc
import concourse.bass as bass
import concourse.mybir as mybir
import concourse.tile as tile
from concourse.bass_utils import run_bass_kernel_spmd
from concourse.masks import make_identity

F32 = mybir.dt.float32
F32R = mybir.dt.float32r
# fp16 (not bf16): same PE rate (1 cyc/row + FWL) but 10 mantissa bits;
# all tensors in this problem are O(10^3) max, far inside fp16 range
BF16 = mybir.dt.float16

B, N, D = 8, 4096, 768
H, DH = 12, 64
P = 128
KT = D // P  # 6 feature tiles
TT = N // P  # 32 token tiles
HP = H // 2  # 6 head pairs (2 heads packed into 128 partitions)
EPS = 1e-12
NPREF = 16  # token tiles whose transposes are hoisted into phase C/D gaps

# bf16 for the final y = x @ W3 matmul (adds ~2.8e-3 absmax-relative error;
# set XCA_BF16_E=0 to run that phase in float32r instead)
BF16_E = os.environ.get("XCA_BF16_E", "1") == "1"

if os.environ.get("BASS_LDW_OPT", "0") == "1":
    # Allow walrus to dedup back-to-back LDWEIGHTS with identical sources
    # (bass passes --enable-ldw-opt=false by default). Loop orders below are
    # arranged so consecutive matmuls share their stationary operand.
    import concourse.bass_utils as _bu

    if not getattr(_bu, "_ldw_opt_patched", False):
        _orig_run_command = _bu.run_command

        def _run_command_ldw(argv, **kwargs):
            argv = [
                "--enable-ldw-opt=true" if a == "--enable-ldw-opt=false" else a
                for a in argv
            ]
            return _orig_run_command(argv, **kwargs)

        _bu.run_command = _run_command_ldw
        _bu._ldw_opt_patched = True


def build_nc():
    nc = bacc.Bacc("TRN2", target_bir_lowering=False, debug=False)

    x_d = nc.dram_tensor("x", (N, D), F32, kind="ExternalInput")
    wqkv_d = nc.dram_tensor("wqkv", (D, 3 * D), F32, kind="ExternalInput")
    temp_d = nc.dram_tensor("temp", (H,), F32, kind="ExternalInput")
    wout_d = nc.dram_tensor("wout", (D, D), F32, kind="ExternalInput")
    bout_d = nc.dram_tensor("bout", (D,), F32, kind="ExternalInput")
    y_d = nc.dram_tensor("y", (N, D), F32, kind="ExternalOutput")

    with tile.TileContext(nc) as tc:
        _emit(tc, nc, x_d, wqkv_d, temp_d, wout_d, bout_d, y_d)
    nc.compile()
    return nc


def _emit(tc, nc, x_d, wqkv_d, temp_d, wout_d, bout_d, y_d):
    from contextlib import ExitStack

    E_DT = BF16

    ctx = ExitStack()
    with ctx:
        # ---------------- pools ----------------
        persist = ctx.enter_context(tc.tile_pool(name="persist", bufs=1))
        bigpool = ctx.enter_context(tc.tile_pool(name="bigpool", bufs=1))
        xbpool = ctx.enter_context(tc.tile_pool(name="xbpool", bufs=6))
        small = ctx.enter_context(tc.tile_pool(name="small", bufs=1))
        tmppool = ctx.enter_context(tc.tile_pool(name="tmppool", bufs=2))
        ypool = ctx.enter_context(tc.tile_pool(name="ypool", bufs=3))
        xtpool = ctx.enter_context(tc.tile_pool(name="xtpool", bufs=NPREF + 4))
        abd_pool = ctx.enter_context(tc.tile_pool(name="abd", bufs=6))

        wqk_sb = persist.tile([P, KT, 2 * D], F32R)  # Wqkv[:, :1536]
        wqk_bf = persist.tile([P, KT, 2 * D], BF16)  # bf16 copy for C @ [Wq|Wk]
        mq_sb = persist.tile([P, KT, D], F32R)  # C @ Wq (later scaled by 1/nq)
        w2_sb = persist.tile([P, KT, D], E_DT)  # blockdiag(A) @ Wout
        wvt_sb = persist.tile([P, KT, D], E_DT)  # Wv^T
        w3_bf = persist.tile([P, KT, D], E_DT)  # W3 in phase-E dtype

        # prime the x stream before any other SWDGE work so the PE starts
        # as early as possible
        xb_head = []
        for t in range(6):
            xb0 = xbpool.tile([P, D], BF16, tag="xb", name="xb")
            nc.gpsimd.dma_start(xb0, x_d[t * P : (t + 1) * P, :])
            xb_head.append(xb0)

        # memsets reject f32r at codegen: build f32 and DVE-cast
        ident32 = small.tile([P, P], F32)
        make_identity(nc, ident32)
        ident = small.tile([P, P], F32R)
        nc.vector.tensor_copy(ident, ident32)
        identb = small.tile([P, P], BF16)
        nc.vector.tensor_copy(identb, ident32)
        ones32 = small.tile([P, P], F32)
        nc.vector.memset(ones32, 1.0)
        ones = small.tile([P, P], F32R)
        nc.vector.tensor_copy(ones, ones32)
        temp_sb = small.tile([P, H], F32)
        nc.gpsimd.dma_start(temp_sb, temp_d[None, :].to_broadcast((P, H)))
        bout_sb = small.tile([P, D], F32)
        nc.gpsimd.dma_start(bout_sb, bout_d[None, :].to_broadcast((P, D)))
        # s / rvec are replicated across all 128 partitions (the norm matmul
        # uses an all-ones [P,P] lhsT, so every output partition holds the sum)
        s_sb = small.tile([P, 2 * D], F32)  # [1/max(nq,eps) | 1/max(nk,eps)]
        rvec = small.tile([P, D], F32)  # temp_h / nk

        c_sb = bigpool.tile([P, KT, D], BF16, tag="big")

        def load_x_bf16(t):
            """One token tile of x, cast fp32->bf16 during the DMA (SWDGE)."""
            xb = xbpool.tile([P, D], BF16, tag="xb", name="xb")
            nc.gpsimd.dma_start(xb, x_d[t * P : (t + 1) * P, :])
            return xb

        # ------------- phase A: C = x^T x in bf16, upper block-triangle -----
        # row-block i covers cols [128*i, 768): 8 matmuls per token tile,
        # exactly 8 PSUM banks
        with tc.tile_pool(name="psC", bufs=1, space="PSUM") as psC:
            cps = [
                psC.tile([P, D - 128 * i], F32, name=f"cps{i}") for i in range(KT)
            ]
            for t in range(TT):
                xb = xb_head[t] if t < 6 else load_x_bf16(t)
                for i in range(KT):
                    w = D - 128 * i
                    for lo in range(0, w, 512):
                        hi = min(lo + 512, w)
                        nc.tensor.matmul(
                            cps[i][:, lo:hi],
                            xb[:, i * P : (i + 1) * P],
                            xb[:, 128 * i + lo : 128 * i + hi],
                            start=(t == 0),
                            stop=(t == TT - 1),
                        )
            for i in range(KT):
                nc.vector.tensor_copy(c_sb[:, i, 128 * i : D], cps[i])

        # weight loads (scalar HWDGE queue; does not block the x stream)
        nc.scalar.dma_start(
            wqk_sb,
            wqkv_d[:, 0 : 2 * D].rearrange("(ko p) c -> p ko c", p=P).bitcast(F32R),
        )
        for k in range(KT):
            nc.vector.tensor_copy(wqk_bf[:, k, :], wqk_sb[:, k, :])

        # shared PSUM pool for all 128x128 PE transposes (phases C..E)
        psTP = ctx.enter_context(tc.tile_pool(name="psTP", bufs=3, space="PSUM"))

        # mirror the lower block-triangle: block(j,i) = block(i,j)^T
        for i in range(KT):
            for j in range(i + 1, KT):
                tpm = psTP.tile([P, P], BF16, tag="tp", name="tpm")
                nc.tensor.transpose(tpm, c_sb[:, i, j * P : (j + 1) * P], identb)
                nc.vector.tensor_copy(c_sb[:, j, i * P : (i + 1) * P], tpm)

        # phase-E prefetch: transpose the first NPREF token tiles now, so the
        # PE has work during the DVE/ACT-heavy normalization phase
        xtt_tiles = {}

        def transpose_tile(t):
            xb = load_x_bf16(t)
            xtt = xtpool.tile([P, KT, P], BF16, tag="xtt", name="xtt")
            for k in range(KT):
                tpe = psTP.tile([P, P], BF16, tag="tp", name="tpe")
                nc.tensor.transpose(tpe, xb[:, k * P : (k + 1) * P], identb)
                nc.vector.tensor_copy(xtt[:, k, :], tpe)
            return xtt

        for tpre in range(NPREF):
            xtt_tiles[tpre] = transpose_tile(tpre)

        # ---------------- phase C: Mqk = C @ [Wq|Wk], norms -----------------
        with tc.tile_pool(name="psMQ", bufs=2, space="PSUM") as psMQ, tc.tile_pool(
            name="psN", bufs=1, space="PSUM"
        ) as psN:
            nrm_ps = psN.tile([P, 2 * D], F32)  # [nq^2 | nk^2], replicated
            for f in range(KT):
                mk_tmp = tmppool.tile([P, D], F32R, tag="mk", name="mk_tmp")
                for nch in range(3):
                    pmq = psMQ.tile([P, 512], F32, tag="pmq", name="pmq")
                    for k in range(KT):
                        nc.tensor.matmul(
                            pmq,
                            c_sb[:, k, f * P : (f + 1) * P],
                            wqk_bf[:, k, nch * 512 : (nch + 1) * 512],
                            start=(k == 0),
                            stop=(k == KT - 1),
                        )
                    if nch == 0:
                        nc.vector.tensor_copy(mq_sb[:, f, 0:512], pmq)
                    elif nch == 1:
                        nc.vector.tensor_copy(mq_sb[:, f, 512:768], pmq[:, 0:256])
                        nc.vector.tensor_copy(mk_tmp[:, 0:256], pmq[:, 256:512])
                    else:
                        nc.vector.tensor_copy(mk_tmp[:, 256:768], pmq)
                # norm partials: nq_j = sum_f Wq[f,j]*Mq[f,j] (and nk likewise)
                wt = tmppool.tile([P, 2 * D], F32R, tag="wt", name="wt")
                nc.vector.tensor_mul(wt[:, 0:D], wqk_sb[:, f, 0:D], mq_sb[:, f, :])
                nc.vector.tensor_mul(wt[:, D : 2 * D], wqk_sb[:, f, D : 2 * D], mk_tmp)
                for lo in range(0, 2 * D, 512):
                    nc.tensor.matmul(
                        nrm_ps[:, lo : lo + 512],
                        ones,
                        wt[:, lo : lo + 512],
                        start=(f == 0),
                        stop=(f == KT - 1),
                    )
            # s = 1 / max(sqrt(nrm2), eps)
            nc.vector.tensor_copy(s_sb, nrm_ps)
        nc.scalar.sqrt(s_sb, s_sb)
        nc.vector.tensor_scalar_max(s_sb, s_sb, EPS)
        nc.vector.reciprocal(s_sb, s_sb)

        # rvec[h*64+p] = temp[h] * s_k[h*64+p]   (replicated on all partitions)
        rv3 = rvec.rearrange("o (h e) -> o h e", h=H)
        nc.vector.tensor_mul(
            rv3,
            s_sb[:, D : 2 * D].rearrange("o (h e) -> o h e", h=H),
            temp_sb[:, :, None].to_broadcast((P, H, DH)),
        )
        # scale Wk in place by rvec (rows=f, cols=(h,p)); Mq in place by 1/nq
        wk_view = wqk_sb[:, :, D : 2 * D]
        nc.vector.tensor_mul(
            wk_view, wk_view, rvec[:, None, :].to_broadcast((P, KT, D))
        )
        mq_view = mq_sb[:, :, :]
        nc.vector.tensor_mul(
            mq_view, mq_view, s_sb[:, None, 0:D].to_broadcast((P, KT, D))
        )



        # ---------------- phase D1: logits G, softmax -----------------------
        # one 128-wide matmul per head PAIR: off-diagonal 64x64 blocks are
        # cross-head garbage that is simply never read
        with tc.tile_pool(name="psG", bufs=1, space="PSUM") as psG:
            gps = [psG.tile([P, 2, P], F32, name=f"gps{i}") for i in range(3)]
            for hp in range(HP):
                out = gps[hp // 2][:, hp % 2, :]
                for f in range(KT):
                    nc.tensor.matmul(
                        out,
                        wqk_sb[:, f, D + hp * P : D + (hp + 1) * P].bitcast(F32),
                        mq_sb[:, f, hp * P : (hp + 1) * P].bitcast(F32),
                        start=(f == 0),
                        stop=(f == KT - 1),
                    )
            # per-head softmax written into the block-diag tile a_bd =
            # [[A_even, 0], [0, A_odd]]; one transpose per pair then gives
            # blockdiag(A_e^T, A_o^T) for the W2 matmul (base-0, K=128)
            abd_tiles = []
            for hp in range(HP):
                a_bd = abd_pool.tile([P, P], F32, tag="a_bd", name="a_bd")
                nc.vector.memset(a_bd, 0.0)
                for half in range(2):
                    lo64 = half * 64
                    g_view = gps[hp // 2][lo64 : lo64 + 64, hp % 2, lo64 : lo64 + 64]
                    negmax = small.tile([P, 1], F32, tag="negmax", name="negmax", bufs=4)
                    sumexp = small.tile([P, 1], F32, tag="sumexp", name="sumexp", bufs=4)
                    rec = small.tile([P, 1], F32, tag="rec", name="rec", bufs=4)
                    nm, se, rc = (
                        negmax[lo64 : lo64 + 64],
                        sumexp[lo64 : lo64 + 64],
                        rec[lo64 : lo64 + 64],
                    )
                    a_blk = a_bd[lo64 : lo64 + 64, lo64 : lo64 + 64]
                    nc.vector.tensor_reduce(
                        nm, g_view, axis=mybir.AxisListType.X,
                        op=mybir.AluOpType.max, negate=True,
                    )
                    nc.scalar.activation(
                        a_blk, g_view, mybir.ActivationFunctionType.Exp,
                        bias=nm, accum_out=se,
                    )
                    nc.vector.reciprocal(rc, se)
                    nc.vector.tensor_scalar_mul(a_blk, a_blk, rc)
                abd_tiles.append(a_bd)

        # ---------------- phase D2: W2 = blockdiag(A) @ Wout ----------------
        with tc.tile_pool(name="psW2", bufs=2, space="PSUM") as psW2:
            for hp in range(HP):
                tps = psTP.tile([P, P], F32, tag="tp", name="tps")
                nc.tensor.transpose(tps, abd_tiles[hp], ident32)
                at_bd = tmppool.tile([P, P], F32R, tag="at_bd", name="at_bd")
                nc.vector.tensor_copy(at_bd, tps)
                wout_pair = tmppool.tile([P, D], F32R, tag="woutp", name="wout_pair")
                nc.scalar.dma_start(
                    wout_pair, wout_d[hp * P : (hp + 1) * P, :].bitcast(F32R)
                )
                w2ps = psW2.tile([P, D], F32, tag="w2ps", name="w2ps")
                for lo, hi in ((0, 512), (512, 768)):
                    nc.tensor.matmul(
                        w2ps[:, lo:hi], at_bd, wout_pair[:, lo:hi],
                        start=True, stop=True,
                    )
                nc.vector.tensor_copy(w2_sb[:, hp, :], w2ps)
            # Wv^T via PE transposes (Wv streamed in 128-col stripes, cast to
            # the phase-E dtype during the DVE copy off PSUM)
            for ci in range(KT):
                wv_st = tmppool.tile([P, KT, P], BF16, tag="wvst", name="wv_st")
                nc.gpsimd.dma_start(
                    wv_st,
                    wqkv_d[:, 2 * D + ci * P : 2 * D + (ci + 1) * P].rearrange(
                        "(ko p) c -> p ko c", p=P
                    ),
                )
                for kf in range(KT):
                    t2 = psTP.tile([P, P], BF16, tag="tp", name="t2")
                    nc.tensor.transpose(t2, wv_st[:, kf, :], identb)
                    nc.vector.tensor_copy(wvt_sb[:, ci, kf * P : (kf + 1) * P], t2)

        # ---------------- phase D3: W3 = Wv @ W2 (phase-E dtype) ------------
        with tc.tile_pool(name="psW3", bufs=2, space="PSUM") as psW3:
            for fi in range(KT):
                w3ps = psW3.tile([P, D], F32, tag="w3ps", name="w3ps")
                for g in range(KT):
                    for lo, hi in ((0, 512), (512, 768)):
                        nc.tensor.matmul(
                            w3ps[:, lo:hi],
                            wvt_sb[:, g, fi * P : (fi + 1) * P],
                            w2_sb[:, g, lo:hi],
                            start=(g == 0),
                            stop=(g == KT - 1),
                        )
                nc.vector.tensor_copy(w3_bf[:, fi, :], w3ps)

        # ---------------- phase E: y = x @ W3 + bout ------------------------
        with tc.tile_pool(name="psY", bufs=2, space="PSUM") as psY:
            for t in range(TT):
                xtt = xtt_tiles.pop(t, None)
                if xtt is None:
                    xtt = transpose_tile(t)
                yps = psY.tile([P, D], F32, tag="yps", name="yps")
                for k in range(KT):
                    for lo, hi in ((0, 512), (512, 768)):
                        nc.tensor.matmul(
                            yps[:, lo:hi],
                            xtt[:, k, :],
                            w3_bf[:, k, lo:hi],
                            start=(k == 0),
                            stop=(k == KT - 1),
                        )
                ysb = ypool.tile([P, D], F32, tag="ysb", name="ysb")
                nc.vector.tensor_add(ysb, yps, bout_sb)
                nc.sync.dma_start(y_d[t * P : (t + 1) * P, :], ysb)


_NC_CACHE = {}


def _get_nc():
    if "nc" not in _NC_CACHE:
        _NC_CACHE["nc"] = build_nc()
    return _NC_CACHE["nc"]


def kernel_with_results(x, Wqkv, temperature, Wout, bout, **run_kwargs):
    x = np.ascontiguousarray(np.asarray(x, dtype=np.float32))
    Wqkv = np.ascontiguousarray(np.asarray(Wqkv, dtype=np.float32))
    temp = np.ascontiguousarray(np.asarray(temperature, dtype=np.float32).reshape(H))
    Wout = np.ascontiguousarray(np.asarray(Wout, dtype=np.float32))
    bout = np.ascontiguousarray(np.asarray(bout, dtype=np.float32))

    nc = _get_nc()
    in_maps = [
        {"x": x[b], "wqkv": Wqkv, "temp": temp, "wout": Wout, "bout": bout}
        for b in range(B)
    ]
    res = run_bass_kernel_spmd(nc, in_maps, core_ids=list(range(B)), **run_kwargs)
    out = np.stack([r["y"] for r in res.results], axis=0)
    return out, res


def kernel(x, Wqkv, temperature, Wout, bout):
    out, _ = kernel_with_results(x, Wqkv, temperature, Wout, bout)
    return out


# revision 30
# speedup vs baseline: 1.0367x; 1.0147x over previous
"""Cross-covariance attention (XCA) kernel for Trainium2, 8 NeuronCores.

Problem (per batch element b, one per core — data-parallel over B=8):
    qkv = x @ Wqkv;  q,k,v heads of dim 64;  q,k L2-normalized over the
    TOKEN axis;  attn_h = softmax((k_h^T q_h) * temp_h) (64x64, head-local);
    y = concat_h(v_h @ attn_h) @ Wout + bout.

Key algebraic reduction: the attention matrix only depends on the token
covariance C = x^T x (768x768):
    k_h^T q_h = Wk_h^T C Wq_h,   ||q_col_j||^2 = diag(Wq^T C Wq)_j
and the output collapses to a single matmul with a precomputed 768x768
matrix:
    y = x @ W3 + bout,  W3 = Wv @ blockdiag(A_h) @ Wout.

Per core: C = x^T x (contraction over tokens => natural x layout; symmetric,
so only the upper block-triangle is computed and the rest is mirrored with
PE transposes), small 768-scale linear algebra to form W3, then y = x @ W3
(x transposed 128x128-blockwise on the PE on the fly).

Precision strategy (validated numerically): the C / final-matmul paths run
in bf16 (PE 1 cyc/row + fast weight load; softmax smooths C errors to
~3e-5), while the norm/logits path (Mqk, norms, G) stays in
float32r/fp32. PSUM accumulation is always fp32.
"""

import os
import sys

sys.path.insert(0, "/opt/trn_rl_repo")

import numpy as np

import concourse.bacc as bacc
import concourse.bass as bass
import concourse.mybir as mybir
import concourse.tile as tile
from concourse.bass_utils import run_bass_kernel_spmd
from concourse.masks import make_identity

F32 = mybir.dt.float32
F32R = mybir.dt.float32r
# fp16 (not bf16): same PE rate (1 cyc/row + FWL) but 10 mantissa bits;
# all tensors in this problem are O(10^3) max, far inside fp16 range
BF16 = mybir.dt.float16

B, N, D = 8, 4096, 768
H, DH = 12, 64
P = 128
KT = D // P  # 6 feature tiles
TT = N // P  # 32 token tiles
HP = H // 2  # 6 head pairs (2 heads packed into 128 partitions)
EPS = 1e-12
NPREF = 18  # token tiles whose transposes are hoisted into phase C/D gaps

# bf16 for the final y = x @ W3 matmul (adds ~2.8e-3 absmax-relative error;
# set XCA_BF16_E=0 to run that phase in float32r instead)
BF16_E = os.environ.get("XCA_BF16_E", "1") == "1"

if os.environ.get("BASS_LDW_OPT", "0") == "1":
    # Allow walrus to dedup back-to-back LDWEIGHTS with identical sources
    # (bass passes --enable-ldw-opt=false by default). Loop orders below are
    # arranged so consecutive matmuls share their stationary operand.
    import concourse.bass_utils as _bu

    if not getattr(_bu, "_ldw_opt_patched", False):
        _orig_run_command = _bu.run_command

        def _run_command_ldw(argv, **kwargs):
            argv = [
                "--enable-ldw-opt=true" if a == "--enable-ldw-opt=false" else a
                for a in argv
            ]
            return _orig_run_command(argv, **kwargs)

        _bu.run_command = _run_command_ldw
        _bu._ldw_opt_patched = True


def build_nc():
    nc = bacc.Bacc("TRN2", target_bir_lowering=False, debug=False)

    x_d = nc.dram_tensor("x", (N, D), F32, kind="ExternalInput")
    wqkv_d = nc.dram_tensor("wqkv", (D, 3 * D), F32, kind="ExternalInput")
    temp_d = nc.dram_tensor("temp", (H,), F32, kind="ExternalInput")
    wout_d = nc.dram_tensor("wout", (D, D), F32, kind="ExternalInput")
    bout_d = nc.dram_tensor("bout", (D,), F32, kind="ExternalInput")
    y_d = nc.dram_tensor("y", (N, D), F32, kind="ExternalOutput")

    with tile.TileContext(nc) as tc:
        _emit(tc, nc, x_d, wqkv_d, temp_d, wout_d, bout_d, y_d)
    nc.compile()
    return nc


def _emit(tc, nc, x_d, wqkv_d, temp_d, wout_d, bout_d, y_d):
    from contextlib import ExitStack

    E_DT = BF16

    ctx = ExitStack()
    with ctx:
        # ---------------- pools ----------------
        persist = ctx.enter_context(tc.tile_pool(name="persist", bufs=1))
        bigpool = ctx.enter_context(tc.tile_pool(name="bigpool", bufs=1))
        xbpool = ctx.enter_context(tc.tile_pool(name="xbpool", bufs=5))
        small = ctx.enter_context(tc.tile_pool(name="small", bufs=1))
        tmppool = ctx.enter_context(tc.tile_pool(name="tmppool", bufs=2))
        ypool = ctx.enter_context(tc.tile_pool(name="ypool", bufs=3))
        xtpool = ctx.enter_context(tc.tile_pool(name="xtpool", bufs=NPREF + 5))
        abd_pool = ctx.enter_context(tc.tile_pool(name="abd", bufs=6))

        wqk_sb = persist.tile([P, KT, 2 * D], F32R)  # Wqkv[:, :1536]
        wqk_bf = persist.tile([P, KT, 2 * D], BF16)  # bf16 copy for C @ [Wq|Wk]
        mq_sb = persist.tile([P, KT, D], F32R)  # C @ Wq (later scaled by 1/nq)
        w2_sb = persist.tile([P, KT, D], E_DT)  # blockdiag(A) @ Wout
        wvt_sb = persist.tile([P, KT, D], E_DT)  # Wv^T
        w3_bf = persist.tile([P, KT, D], E_DT)  # W3 in phase-E dtype

        # prime the x stream before any other SWDGE work so the PE starts
        # as early as possible
        xb_head = []
        for t in range(6):
            xb0 = xbpool.tile([P, D], BF16, tag="xb", name="xb")
            nc.gpsimd.dma_start(xb0, x_d[t * P : (t + 1) * P, :])
            xb_head.append(xb0)

        # memsets reject f32r at codegen: build f32 and DVE-cast
        ident32 = small.tile([P, P], F32)
        make_identity(nc, ident32)
        ident = small.tile([P, P], F32R)
        nc.vector.tensor_copy(ident, ident32)
        identb = small.tile([P, P], BF16)
        nc.vector.tensor_copy(identb, ident32)
        ones32 = small.tile([P, P], F32)
        nc.vector.memset(ones32, 1.0)
        ones = small.tile([P, P], F32R)
        nc.vector.tensor_copy(ones, ones32)
        temp_sb = small.tile([P, H], F32)
        nc.gpsimd.dma_start(temp_sb, temp_d[None, :].to_broadcast((P, H)))
        bout_sb = small.tile([P, D], F32)
        nc.gpsimd.dma_start(bout_sb, bout_d[None, :].to_broadcast((P, D)))
        # s / rvec are replicated across all 128 partitions (the norm matmul
        # uses an all-ones [P,P] lhsT, so every output partition holds the sum)
        s_sb = small.tile([P, 2 * D], F32)  # [1/max(nq,eps) | 1/max(nk,eps)]
        rvec = small.tile([P, D], F32)  # temp_h / nk

        c_sb = bigpool.tile([P, KT, D], BF16, tag="big")

        def load_x_bf16(t):
            """One token tile of x, cast fp32->bf16 during the DMA (SWDGE)."""
            xb = xbpool.tile([P, D], BF16, tag="xb", name="xb")
            nc.gpsimd.dma_start(xb, x_d[t * P : (t + 1) * P, :])
            return xb

        # ------------- phase A: C = x^T x in bf16, upper block-triangle -----
        # row-block i covers cols [128*i, 768): 8 matmuls per token tile,
        # exactly 8 PSUM banks
        with tc.tile_pool(name="psC", bufs=1, space="PSUM") as psC:
            cps = [
                psC.tile([P, D - 128 * i], F32, name=f"cps{i}") for i in range(KT)
            ]
            for t in range(TT):
                xb = xb_head[t] if t < 6 else load_x_bf16(t)
                for i in range(KT):
                    w = D - 128 * i
                    for lo in range(0, w, 512):
                        hi = min(lo + 512, w)
                        nc.tensor.matmul(
                            cps[i][:, lo:hi],
                            xb[:, i * P : (i + 1) * P],
                            xb[:, 128 * i + lo : 128 * i + hi],
                            start=(t == 0),
                            stop=(t == TT - 1),
                        )
            for i in range(KT):
                nc.vector.tensor_copy(c_sb[:, i, 128 * i : D], cps[i])

        # weight loads (scalar HWDGE queue; does not block the x stream)
        nc.scalar.dma_start(
            wqk_sb,
            wqkv_d[:, 0 : 2 * D].rearrange("(ko p) c -> p ko c", p=P).bitcast(F32R),
        )
        for k in range(KT):
            nc.vector.tensor_copy(wqk_bf[:, k, :], wqk_sb[:, k, :])

        # shared PSUM pool for all 128x128 PE transposes (phases C..E)
        psTP = ctx.enter_context(tc.tile_pool(name="psTP", bufs=3, space="PSUM"))

        # mirror the lower block-triangle: block(j,i) = block(i,j)^T
        for i in range(KT):
            for j in range(i + 1, KT):
                tpm = psTP.tile([P, P], BF16, tag="tp", name="tpm")
                nc.tensor.transpose(tpm, c_sb[:, i, j * P : (j + 1) * P], identb)
                nc.vector.tensor_copy(c_sb[:, j, i * P : (i + 1) * P], tpm)

        # phase-E prefetch: transpose the first NPREF token tiles now, so the
        # PE has work during the DVE/ACT-heavy normalization phase
        xtt_tiles = {}

        def transpose_tile(t):
            xb = load_x_bf16(t)
            xtt = xtpool.tile([P, KT, P], BF16, tag="xtt", name="xtt")
            for k in range(KT):
                tpe = psTP.tile([P, P], BF16, tag="tp", name="tpe")
                nc.tensor.transpose(tpe, xb[:, k * P : (k + 1) * P], identb)
                nc.vector.tensor_copy(xtt[:, k, :], tpe)
            return xtt

        for tpre in range(9):
            xtt_tiles[tpre] = transpose_tile(tpre)

        # Wv^T does not depend on anything downstream: hoist its stripes and
        # transposes here so the PE has work during the normalization chain
        for ci in range(KT):
            wv_st = tmppool.tile([P, KT, P], BF16, tag="wvst", name="wv_st")
            nc.gpsimd.dma_start(
                wv_st,
                wqkv_d[:, 2 * D + ci * P : 2 * D + (ci + 1) * P].rearrange(
                    "(ko p) c -> p ko c", p=P
                ),
            )
            for kf in range(KT):
                t2 = psTP.tile([P, P], BF16, tag="tp", name="t2")
                nc.tensor.transpose(t2, wv_st[:, kf, :], identb)
                nc.vector.tensor_copy(wvt_sb[:, ci, kf * P : (kf + 1) * P], t2)

        # ---------------- phase C: Mqk = C @ [Wq|Wk], norms -----------------
        with tc.tile_pool(name="psMQ", bufs=2, space="PSUM") as psMQ, tc.tile_pool(
            name="psN", bufs=1, space="PSUM"
        ) as psN:
            nrm_ps = psN.tile([P, 2 * D], F32)  # [nq^2 | nk^2], replicated
            for f in range(KT):
                mk_tmp = tmppool.tile([P, D], F32R, tag="mk", name="mk_tmp")
                for nch in range(3):
                    pmq = psMQ.tile([P, 512], F32, tag="pmq", name="pmq")
                    for k in range(KT):
                        nc.tensor.matmul(
                            pmq,
                            c_sb[:, k, f * P : (f + 1) * P],
                            wqk_bf[:, k, nch * 512 : (nch + 1) * 512],
                            start=(k == 0),
                            stop=(k == KT - 1),
                        )
                    if nch == 0:
                        nc.vector.tensor_copy(mq_sb[:, f, 0:512], pmq)
                    elif nch == 1:
                        nc.vector.tensor_copy(mq_sb[:, f, 512:768], pmq[:, 0:256])
                        nc.vector.tensor_copy(mk_tmp[:, 0:256], pmq[:, 256:512])
                    else:
                        nc.vector.tensor_copy(mk_tmp[:, 256:768], pmq)
                # norm partials: nq_j = sum_f Wq[f,j]*Mq[f,j] (and nk likewise)
                wt = tmppool.tile([P, 2 * D], F32R, tag="wt", name="wt")
                nc.vector.tensor_mul(wt[:, 0:D], wqk_sb[:, f, 0:D], mq_sb[:, f, :])
                nc.vector.tensor_mul(wt[:, D : 2 * D], wqk_sb[:, f, D : 2 * D], mk_tmp)
                for lo in range(0, 2 * D, 512):
                    nc.tensor.matmul(
                        nrm_ps[:, lo : lo + 512],
                        ones,
                        wt[:, lo : lo + 512],
                        start=(f == 0),
                        stop=(f == KT - 1),
                    )
            # s = 1 / max(sqrt(nrm2), eps)
            nc.vector.tensor_copy(s_sb, nrm_ps)
        nc.scalar.sqrt(s_sb, s_sb)
        nc.vector.tensor_scalar_max(s_sb, s_sb, EPS)
        nc.vector.reciprocal(s_sb, s_sb)

        # rvec[h*64+p] = temp[h] * s_k[h*64+p]   (replicated on all partitions)
        rv3 = rvec.rearrange("o (h e) -> o h e", h=H)
        nc.vector.tensor_mul(
            rv3,
            s_sb[:, D : 2 * D].rearrange("o (h e) -> o h e", h=H),
            temp_sb[:, :, None].to_broadcast((P, H, DH)),
        )
        # scale Wk in place by rvec (rows=f, cols=(h,p)); Mq in place by 1/nq.
        # One op per k-tile so the G matmuls can start as soon as tile 0 is
        # scaled instead of waiting for the full 128x6x768 op.
        for k in range(KT):
            wk_k = wqk_sb[:, k, D : 2 * D]
            nc.vector.tensor_mul(wk_k, wk_k, rvec)
            mq_k = mq_sb[:, k, :]
            nc.vector.tensor_mul(mq_k, mq_k, s_sb[:, 0:D])

        for tpre in range(9, NPREF):
            xtt_tiles[tpre] = transpose_tile(tpre)



        # ---------------- phase D1: logits G, softmax -----------------------
        # one 128-wide matmul per head PAIR: off-diagonal 64x64 blocks are
        # cross-head garbage that is simply never read
        with tc.tile_pool(name="psG", bufs=1, space="PSUM") as psG:
            gps = [psG.tile([P, 2, P], F32, name=f"gps{i}") for i in range(3)]
            for hp in range(HP):
                out = gps[hp // 2][:, hp % 2, :]
                for f in range(KT):
                    nc.tensor.matmul(
                        out,
                        wqk_sb[:, f, D + hp * P : D + (hp + 1) * P].bitcast(F32),
                        mq_sb[:, f, hp * P : (hp + 1) * P].bitcast(F32),
                        start=(f == 0),
                        stop=(f == KT - 1),
                    )
            # per-head softmax written into the block-diag tile a_bd =
            # [[A_even, 0], [0, A_odd]]; one transpose per pair then gives
            # blockdiag(A_e^T, A_o^T) for the W2 matmul (base-0, K=128)
            abd_tiles = []
            for hp in range(HP):
                a_bd = abd_pool.tile([P, P], F32, tag="a_bd", name="a_bd")
                nc.vector.memset(a_bd, 0.0)
                for half in range(2):
                    lo64 = half * 64
                    g_view = gps[hp // 2][lo64 : lo64 + 64, hp % 2, lo64 : lo64 + 64]
                    negmax = small.tile([P, 1], F32, tag="negmax", name="negmax", bufs=4)
                    sumexp = small.tile([P, 1], F32, tag="sumexp", name="sumexp", bufs=4)
                    rec = small.tile([P, 1], F32, tag="rec", name="rec", bufs=4)
                    nm, se, rc = (
                        negmax[lo64 : lo64 + 64],
                        sumexp[lo64 : lo64 + 64],
                        rec[lo64 : lo64 + 64],
                    )
                    a_blk = a_bd[lo64 : lo64 + 64, lo64 : lo64 + 64]
                    nc.vector.tensor_reduce(
                        nm, g_view, axis=mybir.AxisListType.X,
                        op=mybir.AluOpType.max, negate=True,
                    )
                    nc.scalar.activation(
                        a_blk, g_view, mybir.ActivationFunctionType.Exp,
                        bias=nm, accum_out=se,
                    )
                    nc.vector.reciprocal(rc, se)
                    nc.vector.tensor_scalar_mul(a_blk, a_blk, rc)
                abd_tiles.append(a_bd)

        # ---------------- phase D2: W2 = blockdiag(A) @ Wout ----------------
        with tc.tile_pool(name="psW2", bufs=2, space="PSUM") as psW2:
            for hp in range(HP):
                tps = psTP.tile([P, P], F32, tag="tp", name="tps")
                nc.tensor.transpose(tps, abd_tiles[hp], ident32)
                at_bd = tmppool.tile([P, P], F32R, tag="at_bd", name="at_bd")
                nc.vector.tensor_copy(at_bd, tps)
                wout_pair = tmppool.tile([P, D], F32R, tag="woutp", name="wout_pair")
                nc.scalar.dma_start(
                    wout_pair, wout_d[hp * P : (hp + 1) * P, :].bitcast(F32R)
                )
                w2ps = psW2.tile([P, D], F32, tag="w2ps", name="w2ps")
                for lo, hi in ((0, 512), (512, 768)):
                    nc.tensor.matmul(
                        w2ps[:, lo:hi], at_bd, wout_pair[:, lo:hi],
                        start=True, stop=True,
                    )
                nc.vector.tensor_copy(w2_sb[:, hp, :], w2ps)

        # ---------------- phase D3: W3 = Wv @ W2 (phase-E dtype) ------------
        with tc.tile_pool(name="psW3", bufs=2, space="PSUM") as psW3:
            for fi in range(KT):
                w3ps = psW3.tile([P, D], F32, tag="w3ps", name="w3ps")
                for g in range(KT):
                    for lo, hi in ((0, 512), (512, 768)):
                        nc.tensor.matmul(
                            w3ps[:, lo:hi],
                            wvt_sb[:, g, fi * P : (fi + 1) * P],
                            w2_sb[:, g, lo:hi],
                            start=(g == 0),
                            stop=(g == KT - 1),
                        )
                nc.vector.tensor_copy(w3_bf[:, fi, :], w3ps)

        # ---------------- phase E: y = x @ W3 + bout ------------------------
        with tc.tile_pool(name="psY", bufs=2, space="PSUM") as psY:
            for t in range(TT):
                xtt = xtt_tiles.pop(t, None)
                if xtt is None:
                    xtt = transpose_tile(t)
                yps = psY.tile([P, D], F32, tag="yps", name="yps")
                for k in range(KT):
                    for lo, hi in ((0, 512), (512, 768)):
                        nc.tensor.matmul(
                            yps[:, lo:hi],
                            xtt[:, k, :],
                            w3_bf[:, k, lo:hi],
                            start=(k == 0),
                            stop=(k == KT - 1),
                        )
                ysb = ypool.tile([P, D], F32, tag="ysb", name="ysb")
                nc.vector.tensor_add(ysb, yps, bout_sb)
                nc.sync.dma_start(y_d[t * P : (t + 1) * P, :], ysb)


_NC_CACHE = {}


def _get_nc():
    if "nc" not in _NC_CACHE:
        _NC_CACHE["nc"] = build_nc()
    return _NC_CACHE["nc"]


def kernel_with_results(x, Wqkv, temperature, Wout, bout, **run_kwargs):
    x = np.ascontiguousarray(np.asarray(x, dtype=np.float32))
    Wqkv = np.ascontiguousarray(np.asarray(Wqkv, dtype=np.float32))
    temp = np.ascontiguousarray(np.asarray(temperature, dtype=np.float32).reshape(H))
    Wout = np.ascontiguousarray(np.asarray(Wout, dtype=np.float32))
    bout = np.ascontiguousarray(np.asarray(bout, dtype=np.float32))

    nc = _get_nc()
    in_maps = [
        {"x": x[b], "wqkv": Wqkv, "temp": temp, "wout": Wout, "bout": bout}
        for b in range(B)
    ]
    res = run_bass_kernel_spmd(nc, in_maps, core_ids=list(range(B)), **run_kwargs)
    out = np.stack([r["y"] for r in res.results], axis=0)
    return out, res


def kernel(x, Wqkv, temperature, Wout, bout):
    out, _ = kernel_with_results(x, Wqkv, temperature, Wout, bout)
    return out
